# revision 1
# baseline (speedup 1.0000x reference)
"""Deformable conv Bass kernel builder for TRN2 (8-core data-parallel over batch)."""
import numpy as np
import bass_rust
import concourse.bass as bass
import concourse.bacc as bacc
import concourse.bass_isa as bass_isa
import concourse.mybir as mybir
import concourse.tile as tile
from concourse import masks

F32 = mybir.dt.float32
I32 = mybir.dt.int32
I16 = mybir.dt.int16
AOP = mybir.AluOpType
AF = mybir.ActivationFunctionType

C = 64
H = W = 128
HW = H * W
ALPHA = 129.0 / 127.0
MLO = 2
XB = YB = 134
NE = XB * YB              # 17956
NCHUNK = 512              # pixels per chunk = 4 rows
NROWS_CHUNK = 4
NCHUNKS = HW // NCHUNK    # 32
QROWS = 8                 # rows per idx-group ("quarter")
NQ = H // QROWS           # 16
CHUNKS_PER_Q = QROWS // NROWS_CHUNK  # 2
KP = 5
STAGE_ROWS = 34


def _apv(base_ap, dims):
    """Copy of base_ap with explicit [step,count] dims (first = partition)."""
    ap = base_ap.copy()
    ap.ap = bass_rust.VecI64Pair([list(d) for d in dims])
    return ap


def ap_gather_direct(nc, out_ap, in_ap, idxs_ap, channels, num_elems, d, num_idxs):
    g = nc.gpsimd
    _in = g.lower_ap(in_ap, for_isa=True)
    _idx = g.lower_ap(idxs_ap, for_isa=True)
    _out = g.lower_ap(out_ap, for_isa=True)
    return g.add_instruction(
        bass_isa.InstAPGather(
            name=f"I-{nc.next_id()}",
            ins=[_in, _idx],
            outs=[_out],
            _channels=channels,
            _num_elems=num_elems,
            _d=d,
            _num_idxs=num_idxs,
        )
    )


def host_prep(x_b, w_off, b_off, w_reg, b_reg):
    lin = np.linspace(-1, 1, 3).astype(np.float64)
    yy, xx = np.meshgrid(np.arange(H, dtype=np.float64), np.arange(W, dtype=np.float64), indexing="ij")
    grid = np.zeros((18, HW), np.float32)
    for m in range(9):
        pnx, pny = lin[m % 3], lin[m // 3]
        grid[2 * m] = (ALPHA * (xx.ravel() + pnx + np.float64(b_off[2 * m]))).astype(np.float32)
        grid[2 * m + 1] = (ALPHA * (yy.ravel() + pny + np.float64(b_off[2 * m + 1]))).astype(np.float32)
    woff_l = np.ascontiguousarray(w_off.transpose(2, 3, 1, 0).reshape(9, 64, 18)).astype(np.float32)
    wreg_l = np.zeros((5, 128, 64), np.float32)
    wr = w_reg.reshape(64, 64, 9)
    for t in range(5):
        for h in range(2):
            k = 2 * t + h
            if k < 9:
                wreg_l[t, h * 64:(h + 1) * 64, :] = wr[:, :, k].T
    return {
        "x": np.ascontiguousarray(x_b.reshape(C, HW)).astype(np.float32),
        "grid": grid,
        "woff": woff_l.reshape(9 * 64, 18),
        "wreg": wreg_l.reshape(5 * 128, 64),
        "breg": b_reg.reshape(64, 1).astype(np.float32),
    }


def _declare_io(nc):
    return dict(
        x_d=nc.dram_tensor("x", [C, HW], F32, kind="ExternalInput"),
        grid_d=nc.dram_tensor("grid", [18, HW], F32, kind="ExternalInput"),
        woff_d=nc.dram_tensor("woff", [9 * 64, 18], F32, kind="ExternalInput"),
        wreg_d=nc.dram_tensor("wreg", [5 * 128, 64], F32, kind="ExternalInput"),
        breg_d=nc.dram_tensor("breg", [64, 1], F32, kind="ExternalInput"),
        out_d=nc.dram_tensor("out", [C, HW], F32, kind="ExternalOutput"),
        wtd_d=nc.dram_tensor("wtd", [20, HW], F32),
    )


def build(nc, debug=False, io=None):
    if io is None:
        io = _declare_io(nc)
    x_d, grid_d, woff_d, wreg_d, breg_d, out_d, wtd = (
        io["x_d"], io["grid_d"], io["woff_d"], io["wreg_d"], io["breg_d"],
        io["out_d"], io["wtd_d"])
    if debug:
        dbg_w = nc.dram_tensor("dbg_w", [20, NCHUNK], F32, kind="ExternalOutput")
        dbg_ffl = nc.dram_tensor("dbg_ffl", [18, NCHUNK], F32, kind="ExternalOutput")
        dbg_idx = nc.dram_tensor("dbg_idx", [128, KP * QROWS * 8], I16, kind="ExternalOutput")
        dbg_tb = nc.dram_tensor("dbg_tb", [128, 2 * NCHUNK * 2], F32, kind="ExternalOutput")
        dbg_s = nc.dram_tensor("dbg_s", [128, NCHUNK], F32, kind="ExternalOutput")

    with tile.TileContext(nc) as tc:
        with tc.tile_pool(name="const", bufs=1) as constp:
            woff_s = constp.tile([64, 9 * 18], F32, tag="woff")
            for t in range(9):
                nc.sync.dma_start(out=woff_s[:, t * 18:(t + 1) * 18],
                                  in_=woff_d[t * 64:(t + 1) * 64, :])
            wreg_s = constp.tile([128, 5 * 64], F32, tag="wreg")
            for t in range(5):
                nc.sync.dma_start(out=wreg_s[:, t * 64:(t + 1) * 64],
                                  in_=wreg_d[t * 128:(t + 1) * 128, :])
            breg_s = constp.tile([64, 1], F32, tag="breg")
            nc.sync.dma_start(out=breg_s[:], in_=breg_d[:])
            ident = constp.tile([128, 128], F32, tag="ident")
            masks.make_identity(nc, ident[:])

            x2 = constp.tile([128, 2 * NE], F32, tag="x2")

            # ---------- build X2 ----------
            with tc.tile_pool(name="stage", bufs=2) as stagep:
                n_pieces = (YB + STAGE_ROWS - 1) // STAGE_ROWS
                for pc in range(n_pieces):
                    r0 = pc * STAGE_ROWS
                    nrows = min(STAGE_ROWS, YB - r0)
                    seg = stagep.tile([128, STAGE_ROWS * XB + 1], F32, tag="seg")
                    nc.vector.memset(seg[:], 0.0)
                    ylo = max(r0, MLO + 1)
                    yhi = min(r0 + nrows, MLO + 1 + H)
                    if yhi > ylo:
                        xr0 = ylo - (MLO + 1)
                        cnt = yhi - ylo
                        for half in range(2):
                            dst = seg[half * 64:(half + 1) * 64,
                                      (ylo - r0) * XB + MLO + 1:
                                      (ylo - r0) * XB + MLO + 1 + (cnt - 1) * XB + W]
                            dstv = _apv(dst, [dst.ap[0], [XB, cnt], [1, W]])
                            nc.sync.dma_start(
                                out=dstv,
                                in_=x_d[:, xr0 * W:(xr0 + cnt) * W].rearrange(
                                    "p (r w) -> p r w", r=cnt, w=W))
                    nel = nrows * XB
                    if pc == n_pieces - 1:
                        nel -= 1
                    dst = x2[:, 2 * r0 * XB: 2 * r0 * XB + 2 * nel].rearrange(
                        "p (q s) -> p q s", q=nel, s=2)
                    sseg = seg[:, 0:nel + 1]
                    src = _apv(sseg, [list(sseg.ap[0]), [1, nel], [1, 2]])
                    if pc % 2 == 0:
                        nc.vector.tensor_copy(dst, src)
                    else:
                        nc.scalar.copy(dst, src)
                nc.vector.memset(x2[:, 2 * NE - 2: 2 * NE], 0.0)

            x2v = x2[:].rearrange("p (q s) -> p q s", q=NE, s=2)

            with (
                tc.tile_pool(name="psum_o", bufs=2, space="PSUM") as psum_o_p,
                tc.tile_pool(name="psum_t", bufs=2, space="PSUM") as psum_t_p,
                tc.tile_pool(name="psum_m", bufs=2, space="PSUM") as psum_m_p,
                tc.tile_pool(name="wtq", bufs=1) as wtqp,
                tc.tile_pool(name="wpipe", bufs=1) as wpipe,
                tc.tile_pool(name="pmq", bufs=2) as pmqp,
                tc.tile_pool(name="gath", bufs=1) as gathp,
                tc.tile_pool(name="lerp", bufs=1) as lerpp,
                tc.tile_pool(name="bcst", bufs=2) as bcstp,
                tc.tile_pool(name="outp", bufs=2) as outp,
            ):
                for q in range(NQ):
                    fflq = pmqp.tile([128, QROWS * 18], F32, tag="fflq")
                    wtq = wtqp.tile([20, CHUNKS_PER_Q * NCHUNK], F32, tag="wtq")
                    nc.vector.memset(wtq[:], 0.0)
                    # ---------- pass 1: offset conv + weights + ffl transpose ----------
                    for cc in range(CHUNKS_PER_Q):
                        ch = q * CHUNKS_PER_Q + cc
                        y0 = ch * NROWS_CHUNK
                        p0 = ch * NCHUNK
                        po = psum_o_p.tile([18, NCHUNK], F32, tag="po")
                        for t in range(9):
                            dy, dx = t // 3 - 1, t % 3 - 1
                            base = 2 * ((y0 + dy + MLO + 1) * XB + dx + MLO + 1)
                            rhs = x2[0:64, base: base + 2 * XB * NROWS_CHUNK].rearrange(
                                "p (r q2) -> p r q2", r=NROWS_CHUNK, q2=2 * XB)[:, :, 0:2 * W:2]
                            nc.tensor.matmul(po[:], woff_s[:, t * 18:(t + 1) * 18], rhs,
                                             start=(t == 0), stop=(t == 8))
                        gridc = wpipe.tile([18, NCHUNK], F32, tag="gridc")
                        nc.sync.dma_start(out=gridc[:], in_=grid_d[:, p0:p0 + NCHUNK])
                        qt = wpipe.tile([18, NCHUNK], F32, tag="qt")
                        nc.vector.scalar_tensor_tensor(
                            out=qt[:], in0=po[:], scalar=float(ALPHA), in1=gridc[:],
                            op0=AOP.mult, op1=AOP.add)
                        i32t = wpipe.tile([18, NCHUNK], I32, tag="i32t")
                        nc.scalar.copy(i32t[:], qt[:])
                        ft = wpipe.tile([18, NCHUNK], F32, tag="ft")
                        nc.scalar.copy(ft[:], i32t[:])
                        gtt = wpipe.tile([18, NCHUNK], F32, tag="gtt")
                        nc.vector.tensor_tensor(out=gtt[:], in0=ft[:], in1=qt[:], op=AOP.is_gt)
                        fflt = wpipe.tile([18, NCHUNK], F32, tag="fflt")
                        nc.vector.tensor_tensor(out=fflt[:], in0=ft[:], in1=gtt[:], op=AOP.subtract)
                        wt = wtq[:, cc * NCHUNK:(cc + 1) * NCHUNK]
                        nc.vector.tensor_tensor(out=wt[0:18, :], in0=qt[:], in1=fflt[:], op=AOP.subtract)
                        nc.sync.dma_start(out=wtd[:, p0:p0 + NCHUNK], in_=wt[:])
                        if debug and ch == 0:
                            nc.sync.dma_start(out=dbg_w[:], in_=wt[:])
                            nc.sync.dma_start(out=dbg_ffl[:], in_=fflt[:])
                        for r in range(NROWS_CHUNK):
                            pt = psum_t_p.tile([128, 18], F32, tag="pt")
                            nc.tensor.transpose(pt[:], fflt[:, r * W:(r + 1) * W], ident[0:18, 0:18])
                            yq = cc * NROWS_CHUNK + r
                            nc.scalar.copy(fflq[:, yq * 18:(yq + 1) * 18], pt[:])
                    # ---------- idx pipeline (pixel-major) ----------
                    fcl = pmqp.tile([128, QROWS * 18], F32, tag="fcl")
                    nc.vector.tensor_scalar(out=fcl[:], in0=fflq[:], scalar1=-2.0,
                                            scalar2=130.0, op0=AOP.max, op1=AOP.min)
                    fclv = fcl[:].rearrange("p (y c) -> p y c", y=QROWS, c=18)
                    idxf = pmqp.tile([128, QROWS * 9], F32, tag="idxf")
                    nc.vector.scalar_tensor_tensor(
                        out=idxf[:].rearrange("p (y k) -> p y k", y=QROWS, k=9),
                        in0=fclv[:, :, 1:18:2], scalar=float(XB),
                        in1=fclv[:, :, 0:18:2], op0=AOP.mult, op1=AOP.add)
                    pmqt = pmqp.tile([128, QROWS * 10], I16, tag="pmqt")
                    nc.vector.memset(pmqt[:], 0)
                    nc.vector.tensor_scalar(
                        out=pmqt[:].rearrange("p (y k) -> p y k", y=QROWS, k=10)[:, :, 0:9],
                        in0=idxf[:].rearrange("p (y k) -> p y k", y=QROWS, k=9),
                        scalar1=float((MLO) * XB + MLO), scalar2=None, op0=AOP.add)
                    # wrap: WQ[r, (g, y, k)] = pmqt[16g+r, (y, k)]
                    wq = pmqp.tile([16, 8 * QROWS * 10], I16, tag="wq")
                    for g in range(8):
                        nc.sync.dma_start(
                            out=wq[0:16, g * QROWS * 10:(g + 1) * QROWS * 10],
                            in_=pmqt[16 * g:16 * (g + 1), :])
                    # replicate all-k, then per-half k-split via local strided copies
                    w8a = pmqp.tile([128, 8 * QROWS * 10], I16, tag="w8a")
                    for a in range(8):
                        nc.sync.dma_start(
                            out=w8a[16 * a:16 * (a + 1), :], in_=wq[:])
                    w8q = pmqp.tile([128, KP * QROWS * 8], I16, tag="w8q")
                    for h in range(2):
                        srcb = w8a[64 * h:64 * (h + 1), h:]
                        src = _apv(srcb, [list(srcb.ap[0]), [2, KP], [10, QROWS], [80, 8]])
                        nc.vector.tensor_copy(
                            w8q[64 * h:64 * (h + 1), :].rearrange(
                                "p (t y g) -> p t y g", t=KP, y=QROWS, g=8), src)
                    ballq = pmqp.tile([128, KP * QROWS * 8], I16, tag="ballq")
                    nc.vector.tensor_scalar(out=ballq[:], in0=w8q[:], scalar1=XB,
                                            scalar2=None, op0=AOP.add)
                    if debug and q == 0:
                        nc.sync.dma_start(out=dbg_idx[:], in_=w8q[:])
                    # ---------- pass 2: bcast, gather, lerp, conv ----------
                    for cc in range(CHUNKS_PER_Q):
                        ch = q * CHUNKS_PER_Q + cc
                        p0 = ch * NCHUNK
                        yq0 = cc * NROWS_CHUNK
                        pm = psum_m_p.tile([64, NCHUNK], F32, tag="pm")
                        for t in range(KP):
                            wx = bcstp.tile([128, NCHUNK], F32, tag="wx")
                            wy = bcstp.tile([128, NCHUNK], F32, tag="wy")
                            for hh in range(2):
                                r0w = 4 * t + 2 * hh
                                sx = wtd[r0w:r0w + 1, p0:p0 + NCHUNK]
                                sy = wtd[r0w + 1:r0w + 2, p0:p0 + NCHUNK]
                                nc.sync.dma_start(
                                    out=wx[hh * 64:(hh + 1) * 64, :],
                                    in_=_apv(sx, [[0, 64], [1, NCHUNK]]))
                                nc.scalar.dma_start(
                                    out=wy[hh * 64:(hh + 1) * 64, :],
                                    in_=_apv(sy, [[0, 64], [1, NCHUNK]]))
                            tb = gathp.tile([128, 2 * NCHUNK * 2], F32, tag="tb")
                            idxT = w8q[:, t * QROWS * 8 + yq0 * 8: t * QROWS * 8 + yq0 * 8 + 32]
                            idxB = ballq[:, t * QROWS * 8 + yq0 * 8: t * QROWS * 8 + yq0 * 8 + 32]
                            ap_gather_direct(
                                nc, tb[:, 0:2 * NCHUNK].rearrange("p (n d) -> p n d", n=NCHUNK, d=2),
                                x2v, idxT, 128, NE, 2, NCHUNK)
                            ap_gather_direct(
                                nc, tb[:, 2 * NCHUNK:4 * NCHUNK].rearrange("p (n d) -> p n d", n=NCHUNK, d=2),
                                x2v, idxB, 128, NE, 2, NCHUNK)
                            if debug and ch == 0 and t == 0:
                                nc.sync.dma_start(out=dbg_tb[:], in_=tb[:])
                            # lerp
                            dd = lerpp.tile([128, 2 * NCHUNK], F32, tag="dd")
                            odds = tb[:, 1:4 * NCHUNK:2]
                            evens = tb[:, 0:4 * NCHUNK:2]
                            nc.vector.tensor_tensor(out=dd[:], in0=odds, in1=evens, op=AOP.subtract)
                            wxf = wx[:]
                            wxx = _apv(wxf, [list(wxf.ap[0]), [0, 2], [1, NCHUNK]])
                            nc.vector.tensor_tensor(out=dd[:], in0=dd[:], in1=wxx, op=AOP.mult)
                            nc.vector.tensor_tensor(out=odds, in0=dd[:], in1=evens, op=AOP.add)
                            ht = tb[:, 1:2 * NCHUNK:2]
                            hb = tb[:, 2 * NCHUNK + 1:4 * NCHUNK:2]
                            dv = dd[:, 0:NCHUNK]
                            mv = dd[:, NCHUNK:2 * NCHUNK]
                            nc.vector.tensor_tensor(out=dv, in0=hb, in1=ht, op=AOP.subtract)
                            nc.vector.tensor_tensor(out=mv, in0=dv, in1=wy[:], op=AOP.mult)
                            st = lerpp.tile([128, NCHUNK], F32, tag="st")
                            nc.vector.tensor_tensor(out=st[:], in0=mv, in1=ht, op=AOP.add)
                            if debug and ch == 0 and t == 0:
                                nc.sync.dma_start(out=dbg_s[:], in_=st[:])
                            nc.tensor.matmul(pm[:], wreg_s[:, t * 64:(t + 1) * 64], st[:],
                                             start=(t == 0), stop=(t == KP - 1))
                        osb = outp.tile([64, NCHUNK], F32, tag="osb")
                        nc.scalar.activation(osb[:], pm[:], AF.Identity, bias=breg_s[:, 0:1])
                        nc.sync.dma_start(out=out_d[:, p0:p0 + NCHUNK], in_=osb[:])
    return nc


def build_program(debug=False):
    nc = bacc.Bacc("TRN2", target_bir_lowering=False, debug=False, num_devices=8)
    build(nc, debug=debug)
    nc.compile()
    return nc


# ----------------------------------------------------------------------------
# Harness entry point: full inputs in, full output out. Data-parallel over
# batch: core b processes batch b. Program is built/compiled once per process.
# ----------------------------------------------------------------------------
_CACHE = {}


def _get_program(reps):
    import os
    key = ("prog", reps)
    if key not in _CACHE:
        nc = bacc.Bacc("TRN2", target_bir_lowering=False, debug=False, num_devices=8)
        io = _declare_io(nc)
        for _ in range(reps):
            build(nc, debug=False, io=io)
        nc.compile()
        _CACHE[key] = nc
    return _CACHE[key]


def kernel(x, w_off, b_off, w_reg, b_reg, reps=None):
    import os
    from concourse.bass_utils import run_bass_kernel_spmd
    if reps is None:
        reps = int(os.environ.get("DK_REPS", "1"))
    x = np.asarray(x, np.float32)
    w_off = np.asarray(w_off, np.float32)
    b_off = np.asarray(b_off, np.float32)
    w_reg = np.asarray(w_reg, np.float32)
    b_reg = np.asarray(b_reg, np.float32)
    nc = _get_program(reps)
    in_maps = [host_prep(x[b], w_off, b_off, w_reg, b_reg) for b in range(8)]
    r = run_bass_kernel_spmd(nc, in_maps, list(range(8)))
    out = np.stack([r.results[b]["out"].reshape(64, 128, 128) for b in range(8)])
    return out.astype(np.float32)



# revision 11
# speedup vs baseline: 88.1614x; 88.1614x over previous
"""Deformable conv Bass kernel for TRN2 (8-core data-parallel over batch).

v2: latency-bound fixes over v1 —
 - bf16 x2 (halves gather traffic + SBUF), one combined 1024-idx gather per
   (chunk, tap-pair) instead of two 512-idx gathers
 - per-chunk batched bilinear-weight broadcast (2 big DMAs, prefetchable)
   replacing 20 just-in-time 2KB-packet broadcast DMAs per chunk
 - floor via host-side +64 grid shift (trunc == floor for positives); kills
   the is_gt/sub correction ops on DVE
 - y-lerp on gpsimd; its final add folded into the PE accumulation
   (out += W@ht + W@(wy*(hb-ht)))
 - offset-conv taps row-paired into PE tiles (0,96)/(64,96); main conv at
   col 0 so both can overlap in the array
"""
import os
import numpy as np
import ml_dtypes
import bass_rust
import concourse.bass as bass
import concourse.bacc as bacc
import concourse.bass_isa as bass_isa
import concourse.mybir as mybir
import concourse.tile as tile
from concourse import masks

F32 = mybir.dt.float32
BF16 = mybir.dt.bfloat16
I32 = mybir.dt.int32
I16 = mybir.dt.int16
AOP = mybir.AluOpType
AF = mybir.ActivationFunctionType

C = 64
H = W = 128
HW = H * W
ALPHA = 129.0 / 127.0
MLO = 2
XB = YB = 134
NE = XB * YB              # 17956
NCHUNK = 512              # pixels per chunk = 4 rows
NROWS_CHUNK = 4
NCHUNKS = HW // NCHUNK    # 32
QROWS = 8                 # rows per idx-group
NQ = H // QROWS           # 16
CHUNKS_PER_Q = QROWS // NROWS_CHUNK  # 2
KP = 5
STAGE_ROWS = 34
FSH = 64.0                              # floor shift: trunc(q+64) == floor(q)+64
IDX_OFF = MLO * XB + MLO - 64 * (XB + 1)  # 270 - 8640 = -8370
CLIP_LO = float(-MLO + 64)              # 62
CLIP_HI = float(130 + 64)               # 194
DK_NOZZ = bool(int(os.environ.get("DK_NOZZ", "0")))       # plain zero-fill DMA
DK_WBSIMPLE = bool(int(os.environ.get("DK_WBSIMPLE", "0")))  # per-t wb DMAs
DK_NOTP = bool(int(os.environ.get("DK_NOTP", "0")))       # po at partitions 0-17
DK_GATHER2 = bool(int(os.environ.get("DK_GATHER2", "0")))  # two 512-idx gathers


def _apv(base_ap, dims):
    """Copy of base_ap with explicit [step,count] dims (first = partition)."""
    ap = base_ap.copy()
    ap.ap = bass_rust.VecI64Pair([list(d) for d in dims])
    return ap


def host_prep(x_b, w_off, b_off, w_reg, b_reg):
    lin = np.linspace(-1, 1, 3).astype(np.float64)
    yy, xx = np.meshgrid(np.arange(H, dtype=np.float64), np.arange(W, dtype=np.float64), indexing="ij")
    grid = np.zeros((18, HW), np.float32)
    for m in range(9):
        pnx, pny = lin[m % 3], lin[m // 3]
        grid[2 * m] = (ALPHA * (xx.ravel() + pnx + np.float64(b_off[2 * m])) + FSH).astype(np.float32)
        grid[2 * m + 1] = (ALPHA * (yy.ravel() + pny + np.float64(b_off[2 * m + 1])) + FSH).astype(np.float32)
    woff_l = np.ascontiguousarray(w_off.transpose(2, 3, 1, 0).reshape(9, 64, 18)).astype(np.float32)
    woff2 = woff_l.transpose(1, 0, 2).reshape(64, 9 * 18)
    wreg_l = np.zeros((KP, 128, 64), np.float32)
    wr = w_reg.reshape(64, 64, 9)
    for t in range(KP):
        for h in range(2):
            k = 2 * t + h
            if k < 9:
                wreg_l[t, h * 64:(h + 1) * 64, :] = wr[:, :, k].T
    return {
        "x": np.ascontiguousarray(x_b.reshape(C, HW)).astype(ml_dtypes.bfloat16),
        "grid": grid,
        "woff": woff2.astype(ml_dtypes.bfloat16),
        "wreg": wreg_l.reshape(KP * 128, 64).astype(ml_dtypes.bfloat16),
        "breg": b_reg.reshape(64, 1).astype(np.float32),
    }


def _declare_io(nc):
    return dict(
        x_d=nc.dram_tensor("x", [C, HW], BF16, kind="ExternalInput"),
        grid_d=nc.dram_tensor("grid", [18, HW], F32, kind="ExternalInput"),
        woff_d=nc.dram_tensor("woff", [64, 9 * 18], BF16, kind="ExternalInput"),
        wreg_d=nc.dram_tensor("wreg", [KP * 128, 64], BF16, kind="ExternalInput"),
        breg_d=nc.dram_tensor("breg", [64, 1], F32, kind="ExternalInput"),
        out_d=nc.dram_tensor("out", [C, HW], F32, kind="ExternalOutput"),
        wtd_d=nc.dram_tensor("wtd", [20, HW], BF16),
    )


def build(nc, io=None):
    if io is None:
        io = _declare_io(nc)
    x_d, grid_d, woff_d, wreg_d, breg_d, out_d, wtd = (
        io["x_d"], io["grid_d"], io["woff_d"], io["wreg_d"], io["breg_d"],
        io["out_d"], io["wtd_d"])

    with tile.TileContext(nc) as tc:
        with tc.tile_pool(name="const", bufs=1) as constp:
            woff_s = constp.tile([64, 9 * 18], BF16, tag="woff")
            nc.sync.dma_start(out=woff_s[:], in_=woff_d[:])
            wreg_s = constp.tile([128, KP * 64], BF16, tag="wreg")
            for t in range(KP):
                nc.scalar.dma_start(out=wreg_s[:, t * 64:(t + 1) * 64],
                                    in_=wreg_d[t * 128:(t + 1) * 128, :])
            breg_s = constp.tile([64, 1], F32, tag="breg")
            nc.scalar.dma_start(out=breg_s[:], in_=breg_d[:])
            ident = constp.tile([128, 128], F32, tag="ident")
            masks.make_identity(nc, ident[:])
            # wtd rows 18-19 (pseudo-tap 9) must be finite: zero them once
            zz = constp.tile([2, 2048], BF16, tag="zz")
            nc.vector.memset(zz[:], 0.0)
            if DK_NOZZ:
                for zk in range(HW // 2048):
                    nc.sync.dma_start(out=wtd[18:20, zk * 2048:(zk + 1) * 2048], in_=zz[:])
            else:
                zzv = _apv(zz[:], [list(zz[:].ap[0]), [0, HW // 2048], [1, 2048]])
                nc.sync.dma_start(out=wtd[18:20, :], in_=zzv)

            x2 = constp.tile([128, 2 * NE], BF16, tag="x2")

            # ---------- build X2 (pair-duplicated padded image, bf16) ----------
            with tc.tile_pool(name="stage", bufs=2) as stagep:
                n_pieces = (YB + STAGE_ROWS - 1) // STAGE_ROWS
                for pc in range(n_pieces):
                    r0 = pc * STAGE_ROWS
                    nrows = min(STAGE_ROWS, YB - r0)
                    seg = stagep.tile([128, STAGE_ROWS * XB + 1], BF16, tag="seg")
                    nc.vector.memset(seg[:], 0.0)
                    ylo = max(r0, MLO + 1)
                    yhi = min(r0 + nrows, MLO + 1 + H)
                    if yhi > ylo:
                        xr0 = ylo - (MLO + 1)
                        cnt = yhi - ylo
                        for half in range(2):
                            dst = seg[half * 64:(half + 1) * 64,
                                      (ylo - r0) * XB + MLO + 1:
                                      (ylo - r0) * XB + MLO + 1 + (cnt - 1) * XB + W]
                            dstv = _apv(dst, [dst.ap[0], [XB, cnt], [1, W]])
                            nc.sync.dma_start(
                                out=dstv,
                                in_=x_d[:, xr0 * W:(xr0 + cnt) * W].rearrange(
                                    "p (r w) -> p r w", r=cnt, w=W))
                    nel = nrows * XB
                    if pc == n_pieces - 1:
                        nel -= 1
                    dst = x2[:, 2 * r0 * XB: 2 * r0 * XB + 2 * nel].rearrange(
                        "p (q s) -> p q s", q=nel, s=2)
                    sseg = seg[:, 0:nel + 1]
                    src = _apv(sseg, [list(sseg.ap[0]), [1, nel], [1, 2]])
                    if pc % 2 == 0:
                        nc.vector.tensor_copy(dst, src)
                    else:
                        nc.scalar.copy(dst, src)
                nc.vector.memset(x2[:, 2 * NE - 2: 2 * NE], 0.0)

            x2v = x2[:].rearrange("p (q s) -> p q s", q=NE, s=2)

            with (
                tc.tile_pool(name="psum_o", bufs=2, space="PSUM") as psum_o_p,
                tc.tile_pool(name="psum_t", bufs=2, space="PSUM") as psum_t_p,
                tc.tile_pool(name="psum_m", bufs=2, space="PSUM") as psum_m_p,
                tc.tile_pool(name="wtq", bufs=2) as wtqp,
                tc.tile_pool(name="wpipe", bufs=2) as wpipe,
                tc.tile_pool(name="wb", bufs=2) as wbp,
                tc.tile_pool(name="pmq", bufs=2) as pmqp,
                tc.tile_pool(name="gath", bufs=3) as gathp,
                tc.tile_pool(name="lerp", bufs=3) as lerpp,
                tc.tile_pool(name="gplerp", bufs=3) as gplerpp,
                tc.tile_pool(name="outp", bufs=2) as outp,
            ):
                for q in range(NQ):
                    fflq = pmqp.tile([128, QROWS * 18], F32, tag="fflq")
                    wb_t = [None, None]
                    # ---------- pass 1: offset conv + weights + ffl transpose ----------
                    for cc in range(CHUNKS_PER_Q):
                        ch = q * CHUNKS_PER_Q + cc
                        y0 = ch * NROWS_CHUNK
                        p0 = ch * NCHUNK
                        po = psum_o_p.tile([128, NCHUNK], F32, tag="po")
                        pov = po[0:18, :]
                        for k in range(9):
                            dy, dx = k // 3 - 1, k % 3 - 1
                            base = 2 * ((y0 + dy + MLO + 1) * XB + dx + MLO + 1)
                            rhs = x2[0:64,
                                     base: base + 2 * XB * NROWS_CHUNK].rearrange(
                                "p (r q2) -> p r q2", r=NROWS_CHUNK, q2=2 * XB)[:, :, 0:2 * W:2]
                            nc.tensor.matmul(
                                pov, woff_s[:, k * 18:(k + 1) * 18],
                                rhs, start=(k == 0), stop=(k == 8))
                        gridc = wpipe.tile([18, NCHUNK], F32, tag="gridc")
                        nc.scalar.dma_start(out=gridc[:], in_=grid_d[:, p0:p0 + NCHUNK])
                        # qt = alpha*po + grid64  (shifted coords, always > 0)
                        qt = wpipe.tile([18, NCHUNK], F32, tag="qt")
                        nc.vector.scalar_tensor_tensor(
                            out=qt[:], in0=pov, scalar=float(ALPHA), in1=gridc[:],
                            op0=AOP.mult, op1=AOP.add)
                        i32t = wpipe.tile([18, NCHUNK], I32, tag="i32t")
                        nc.scalar.copy(i32t[:], qt[:])          # trunc == floor (q+64>0)
                        ft = wpipe.tile([18, NCHUNK], F32, tag="ft")
                        nc.scalar.copy(ft[:], i32t[:])
                        wtq = wtqp.tile([18, NCHUNK], BF16, tag="wtq")
                        nc.vector.tensor_tensor(out=wtq[:], in0=qt[:], in1=ft[:],
                                                op=AOP.subtract)
                        nc.sync.dma_start(out=wtd[0:18, p0:p0 + NCHUNK], in_=wtq[:])
                        # batched bilinear-weight broadcast for this chunk:
                        # wb[64h+c, (t, xy, n)] = wtd[4t+2h+xy, p0+n]
                        wbt = wbp.tile([128, KP * 2 * NCHUNK], BF16, tag="wb")
                        for h in range(2):
                            eng = nc.sync if h == 0 else nc.scalar
                            if DK_WBSIMPLE:
                                for t in range(KP):
                                    for s in range(2):
                                        srcv = _apv(wtd[4 * t + 2 * h + s:4 * t + 2 * h + s + 1,
                                                        p0:p0 + NCHUNK],
                                                    [[0, 64], [1, NCHUNK]])
                                        eng.dma_start(
                                            out=wbt[h * 64:(h + 1) * 64,
                                                    (t * 2 + s) * NCHUNK:(t * 2 + s + 1) * NCHUNK],
                                            in_=srcv)
                            else:
                                for s in range(2):
                                    srcv = _apv(wtd[2 * h + s:2 * h + s + 1, p0:p0 + 1],
                                                [[0, 64], [4 * HW, KP], [1, NCHUNK]])
                                    dstb = wbt[h * 64:(h + 1) * 64, s * NCHUNK:]
                                    dstv = _apv(dstb, [list(dstb.ap[0]),
                                                       [2 * NCHUNK, KP], [1, NCHUNK]])
                                    eng.dma_start(out=dstv, in_=srcv)
                        wb_t[cc] = wbt
                        for r in range(NROWS_CHUNK):
                            pt = psum_t_p.tile([128, 18], F32, tag="pt")
                            nc.tensor.transpose(pt[:], ft[:, r * W:(r + 1) * W], ident[0:18, 0:18])
                            yq = cc * NROWS_CHUNK + r
                            nc.scalar.copy(fflq[:, yq * 18:(yq + 1) * 18], pt[:])
                    # ---------- idx pipeline (pixel-major) ----------
                    fcl = pmqp.tile([128, QROWS * 18], F32, tag="fcl")
                    nc.vector.tensor_scalar(out=fcl[:], in0=fflq[:], scalar1=CLIP_LO,
                                            scalar2=CLIP_HI, op0=AOP.max, op1=AOP.min)
                    fclv = fcl[:].rearrange("p (y c) -> p y c", y=QROWS, c=18)
                    idxf = pmqp.tile([128, QROWS * 9], F32, tag="idxf")
                    nc.vector.scalar_tensor_tensor(
                        out=idxf[:].rearrange("p (y k) -> p y k", y=QROWS, k=9),
                        in0=fclv[:, :, 1:18:2], scalar=float(XB),
                        in1=fclv[:, :, 0:18:2], op0=AOP.mult, op1=AOP.add)
                    pmqt = pmqp.tile([128, QROWS * 10], I16, tag="pmqt")
                    nc.vector.memset(pmqt[:], 0)
                    nc.vector.tensor_scalar(
                        out=pmqt[:].rearrange("p (y k) -> p y k", y=QROWS, k=10)[:, :, 0:9],
                        in0=idxf[:].rearrange("p (y k) -> p y k", y=QROWS, k=9),
                        scalar1=float(IDX_OFF), scalar2=None, op0=AOP.add)
                    # wrap: WQ[r, (g, y, k)] = pmqt[16g+r, (y, k)]
                    wq = pmqp.tile([16, 8 * QROWS * 10], I16, tag="wq")
                    for g in range(8):
                        nc.sync.dma_start(
                            out=wq[0:16, g * QROWS * 10:(g + 1) * QROWS * 10],
                            in_=pmqt[16 * g:16 * (g + 1), :])
                    # replicate to all 8 cores
                    w8a = pmqp.tile([128, 8 * QROWS * 10], I16, tag="w8a")
                    for a in range(8):
                        eng = nc.scalar if a % 2 == 0 else nc.sync
                        eng.dma_start(out=w8a[16 * a:16 * (a + 1), :], in_=wq[:])
                    # combined idx: w8c[p, (t, cc, b, y4, g8)], b in {top=+0, bot=+XB}
                    w8c = pmqp.tile([128, KP * 2 * 2 * 32], I16, tag="w8c")
                    for h in range(2):
                        srcb = w8a[64 * h:64 * (h + 1), h:]
                        src = _apv(srcb, [list(srcb.ap[0]), [2, KP], [40, 2], [10, 4], [80, 8]])
                        for b in range(2):
                            dstb = w8c[64 * h:64 * (h + 1), b * 32:]
                            dstv = _apv(dstb, [list(dstb.ap[0]),
                                               [128, KP], [64, 2], [8, 4], [1, 8]])
                            if b == 0:
                                nc.vector.tensor_copy(dstv, src)
                            else:
                                nc.vector.tensor_scalar(out=dstv, in0=src, scalar1=XB,
                                                        scalar2=None, op0=AOP.add)
                    # ---------- pass 2: gather, lerp, conv ----------
                    for cc in range(CHUNKS_PER_Q):
                        ch = q * CHUNKS_PER_Q + cc
                        p0 = ch * NCHUNK
                        wbt = wb_t[cc]
                        pm = psum_m_p.tile([64, NCHUNK], F32, tag="pm")
                        for t in range(KP):
                            tb = gathp.tile([128, 4 * NCHUNK], BF16, tag="tb")
                            if DK_GATHER2:
                                idxT = w8c[:, t * 128 + cc * 64: t * 128 + cc * 64 + 32]
                                idxB = w8c[:, t * 128 + cc * 64 + 32: t * 128 + cc * 64 + 64]
                                nc.gpsimd.ap_gather(
                                    tb[:, 0:2 * NCHUNK].rearrange("p (n d) -> p n d", n=NCHUNK, d=2),
                                    x2v, idxT, 128, NE, 2, NCHUNK)
                                nc.gpsimd.ap_gather(
                                    tb[:, 2 * NCHUNK:4 * NCHUNK].rearrange("p (n d) -> p n d", n=NCHUNK, d=2),
                                    x2v, idxB, 128, NE, 2, NCHUNK)
                            else:
                                idx = w8c[:, t * 128 + cc * 64: t * 128 + cc * 64 + 64]
                                nc.gpsimd.ap_gather(
                                    tb[:].rearrange("p (n d) -> p n d", n=2 * NCHUNK, d=2),
                                    x2v, idx, 128, NE, 2, 2 * NCHUNK)
                            odds = tb[:, 1:4 * NCHUNK:2]
                            evens = tb[:, 0:4 * NCHUNK:2]
                            dd = lerpp.tile([128, 2 * NCHUNK], BF16, tag="dd")
                            nc.vector.tensor_tensor(out=dd[:], in0=odds, in1=evens, op=AOP.subtract)
                            wxf = wbt[:, t * 2 * NCHUNK: t * 2 * NCHUNK + NCHUNK]
                            wxx = _apv(wxf, [list(wxf.ap[0]), [0, 2], [1, NCHUNK]])
                            nc.vector.tensor_tensor(out=dd[:], in0=dd[:], in1=wxx, op=AOP.mult)
                            ab = lerpp.tile([128, 2 * NCHUNK], BF16, tag="ab")
                            nc.vector.tensor_tensor(out=ab[:], in0=evens, in1=dd[:], op=AOP.add)
                            nc.tensor.matmul(pm[:], wreg_s[:, t * 64:(t + 1) * 64],
                                             ab[:, 0:NCHUNK], start=(t == 0), stop=False)
                            dv = gplerpp.tile([128, NCHUNK], BF16, tag="dv")
                            nc.vector.tensor_tensor(out=dv[:], in0=ab[:, NCHUNK:2 * NCHUNK],
                                                    in1=ab[:, 0:NCHUNK], op=AOP.subtract)
                            m = gplerpp.tile([128, NCHUNK], BF16, tag="m")
                            wyv = wbt[:, t * 2 * NCHUNK + NCHUNK: (t + 1) * 2 * NCHUNK]
                            nc.vector.tensor_tensor(out=m[:], in0=dv[:], in1=wyv, op=AOP.mult)
                            nc.tensor.matmul(pm[:], wreg_s[:, t * 64:(t + 1) * 64],
                                             m[:], start=False, stop=(t == KP - 1))
                        osb = outp.tile([64, NCHUNK], F32, tag="osb")
                        nc.scalar.activation(osb[:], pm[:], AF.Identity, bias=breg_s[:, 0:1])
                        eng = nc.sync if cc == 0 else nc.scalar
                        eng.dma_start(out=out_d[:, p0:p0 + NCHUNK], in_=osb[:])
    return nc


# ----------------------------------------------------------------------------
# Harness entry point: full inputs in, full output out. Data-parallel over
# batch: core b processes batch b. Program is built/compiled once per process.
# ----------------------------------------------------------------------------
_CACHE = {}


def _get_program(reps):
    key = ("prog", reps)
    if key not in _CACHE:
        nc = bacc.Bacc("TRN2", target_bir_lowering=False, debug=False, num_devices=8)
        io = _declare_io(nc)
        for _ in range(reps):
            build(nc, io=io)
        nc.compile()
        _CACHE[key] = nc
    return _CACHE[key]


def kernel(x, w_off, b_off, w_reg, b_reg, reps=None):
    import os
    from concourse.bass_utils import run_bass_kernel_spmd
    if reps is None:
        reps = int(os.environ.get("DK_REPS", "1"))
    x = np.asarray(x, np.float32)
    w_off = np.asarray(w_off, np.float32)
    b_off = np.asarray(b_off, np.float32)
    w_reg = np.asarray(w_reg, np.float32)
    b_reg = np.asarray(b_reg, np.float32)
    nc = _get_program(reps)
    in_maps = [host_prep(x[b], w_off, b_off, w_reg, b_reg) for b in range(8)]
    r = run_bass_kernel_spmd(nc, in_maps, list(range(8)))
    out = np.stack([r.results[b]["out"].reshape(64, 128, 128) for b in range(8)])
    return out.astype(np.float32)
